# revision 9
# baseline (speedup 1.0000x reference)
"""Trainium2 Bass kernel for nn_Compiler_58420145160287 (moe_routing).

Strategy: data-parallel over batch B=8 across the 8 NeuronCores (one batch
element per core).  Each core computes, for its batch element:
  - concept head (tt/pol) + pooled mean + context     (fp32 matmuls/reduces)
  - 4-step GRU program generator + softmax weights w  (bf16 matvecs, fp32 math)
  - 8 expert MLPs  Linear(1032->1024)-GELU-Linear(1024->1024)  (bf16 matmuls,
    fp32 PSUM accumulation), combined on the fly with the per-expert scalar
    weights w[v].
The concept-feature columns of W1 are folded into an effective W1 on the host
(x_aug @ W1.T == x @ W1_eff.T + const), so the device contraction is K=1024.
All matmuls run transposed (outputs [feature, token]) so no on-device
transposes of activations are needed; the host transposes the final gathered
outputs.
"""

import os

import numpy as np
import ml_dtypes

B, T, H = 8, 512, 1024
DC = 8
DIN = H + DC
V, L = 8, 4
P = 128
KH = H // P  # 8
MH = H // P  # 8
NCORES = 8

BF16 = ml_dtypes.bfloat16

_CACHE = {}
LAST_RESULTS = None  # test harness introspection (exec_time_ns etc)


def _build_program():
    from contextlib import ExitStack

    import concourse.bass as bass
    import concourse.mybir as mybir
    import concourse.tile as tile
    from concourse import bacc
    from concourse.alu_op_type import AluOpType

    f32 = mybir.dt.float32
    bf16 = mybir.dt.bfloat16
    AF = mybir.ActivationFunctionType
    ts = bass.ts

    nc = bacc.Bacc(
        "TRN2", target_bir_lowering=False, debug=False, num_devices=NCORES
    )

    # ---- DRAM inputs (per-core) ----
    xt_d = nc.dram_tensor("xt", (P, KH, T), f32, kind="ExternalInput")
    w1_d = nc.dram_tensor("w1", (V, P, KH, H), bf16, kind="ExternalInput")
    w2_d = nc.dram_tensor("w2", (V, P, KH, H), bf16, kind="ExternalInput")
    wcf_d = nc.dram_tensor("wcf", (P, KH, DC), f32, kind="ExternalInput")
    bcf_d = nc.dram_tensor("bcf", (DC, 1), f32, kind="ExternalInput")
    wc2hm_d = nc.dram_tensor("wc2hm", (P, KH, H), bf16, kind="ExternalInput")
    wc2ht_d = nc.dram_tensor("wc2ht", (DC, H), bf16, kind="ExternalInput")
    bc2h_d = nc.dram_tensor("bc2h", (1, H), f32, kind="ExternalInput")
    whh_d = nc.dram_tensor("whh", (P, KH, 3 * H), bf16, kind="ExternalInput")
    brz_d = nc.dram_tensor("brz", (1, 2 * H), f32, kind="ExternalInput")
    bhn_d = nc.dram_tensor("bhn", (1, H), f32, kind="ExternalInput")
    bin_d = nc.dram_tensor("bin", (1, H), f32, kind="ExternalInput")
    wpj_d = nc.dram_tensor("wpj", (P, KH, V), f32, kind="ExternalInput")
    bpj_d = nc.dram_tensor("bpj", (1, V), f32, kind="ExternalInput")
    b1c_d = nc.dram_tensor("b1c", (P, V, MH), f32, kind="ExternalInput")
    b2c_d = nc.dram_tensor("b2c", (P, MH, V), f32, kind="ExternalInput")

    # ---- DRAM outputs ----
    outT_o = nc.dram_tensor("outT", (P, MH, T), f32, kind="ExternalOutput")
    lg_o = nc.dram_tensor("lg", (L, V), f32, kind="ExternalOutput")
    cf_o = nc.dram_tensor("cf", (DC, T), f32, kind="ExternalOutput")
    pool_o = nc.dram_tensor("pool", (P, KH), f32, kind="ExternalOutput")

    xt = xt_d.ap()
    w1a = w1_d.ap()
    w2a = w2_d.ap()

    with tile.TileContext(nc) as tc, ExitStack() as ctx:
        pin = ctx.enter_context(tc.tile_pool(name="pin", bufs=1))

        # persistent SBUF tiles
        xbf = pin.tile([P, KH, T], bf16, tag="xbf")
        wcf = pin.tile([P, KH, DC], f32, tag="wcf")
        bcf = pin.tile([DC, 1], f32, tag="bcf")
        wc2ht = pin.tile([DC, H], bf16, tag="wc2ht")
        bc2h = pin.tile([1, H], f32, tag="bc2h")
        brz = pin.tile([1, 2 * H], f32, tag="brz")
        bhn = pin.tile([1, H], f32, tag="bhn")
        bin_ = pin.tile([1, H], f32, tag="bin")
        wpj = pin.tile([P, KH, V], f32, tag="wpj")
        bpj = pin.tile([1, V], f32, tag="bpj")
        b1c = pin.tile([P, V, MH], f32, tag="b1c")
        b2c = pin.tile([P, MH, V], f32, tag="b2c")
        ones1 = pin.tile([1, 1], f32, tag="ones1")
        onesr = pin.tile([1, P], f32, tag="onesr")
        wb = pin.tile([P, V], f32, tag="wb")
        b2mix = pin.tile([P, MH], f32, tag="b2mix")
        hT_f = pin.tile([P, KH], f32, tag="hT_f")
        hT_b = pin.tile([P, KH], bf16, tag="hT_b")
        ctxc = pin.tile([P, KH + 1], bf16, tag="ctxc")
        cfT = pin.tile([DC, T], f32, tag="cfT")
        poolcol = pin.tile([P, KH], f32, tag="poolcol")
        w_work = pin.tile([1, V], f32, tag="w_work")
        w_row = pin.tile([1, V], f32, tag="w_row")
        lgrows = [pin.tile([1, V], f32, tag=f"lg{l}", name=f"lgrow{l}") for l in range(L)]

        nc.vector.memset(ones1, 1.0)
        nc.vector.memset(onesr, 1.0)

        # Pool open order is chosen for LIFO (stack) release:
        #   SBUF stack: pin, w1p, hmp, grus  -> grus released before w2p opens
        #   PSUM stack: ps_a, ps_g           -> ps_g released before ps_b opens
        w1p = ctx.enter_context(tc.tile_pool(name="w1p", bufs=3))
        hmp = ctx.enter_context(tc.tile_pool(name="hmp", bufs=6))
        ps_a = ctx.enter_context(tc.tile_pool(name="ps_a", bufs=2, space="PSUM"))
        ps_g_cm = tc.tile_pool(name="ps_g", bufs=2, space="PSUM")
        ps_g = ps_g_cm.__enter__()
        grus_cm = tc.tile_pool(name="grus", bufs=1)
        grus = grus_cm.__enter__()

        # GRU-lifetime tiles (freed before phase B)
        whh = grus.tile([P, KH, 3 * H], bf16, tag="whh")
        h_row = grus.tile([1, H], f32, tag="h_row")
        r_row = grus.tile([1, H], f32, tag="r_row")
        z_row = grus.tile([1, H], f32, tag="z_row")
        n_row = grus.tile([1, H], f32, tag="n_row")
        t_row = grus.tile([1, H], f32, tag="t_row")

        # persistent loads
        nc.sync.dma_start(out=whh, in_=whh_d.ap())
        nc.sync.dma_start(out=wcf, in_=wcf_d.ap())
        nc.sync.dma_start(out=bcf, in_=bcf_d.ap())
        nc.sync.dma_start(out=wc2ht, in_=wc2ht_d.ap())
        nc.sync.dma_start(out=bc2h, in_=bc2h_d.ap())
        nc.sync.dma_start(out=brz, in_=brz_d.ap())
        nc.sync.dma_start(out=bhn, in_=bhn_d.ap())
        nc.sync.dma_start(out=bin_, in_=bin_d.ap())
        nc.sync.dma_start(out=wpj, in_=wpj_d.ap())
        nc.sync.dma_start(out=bpj, in_=bpj_d.ap())
        nc.sync.dma_start(out=b1c, in_=b1c_d.ap())
        nc.sync.dma_start(out=b2c, in_=b2c_d.ap())

        # ---------------- prologue ----------------
        xtf = w1p.tile([P, KH, T], f32, tag="w1", name="xtf")
        nc.sync.dma_start(out=xtf, in_=xt)
        wc2hm = w1p.tile([P, KH, H], bf16, tag="w1", name="wc2hm")
        nc.sync.dma_start(out=wc2hm, in_=wc2hm_d.ap())
        nc.vector.tensor_copy(out=xbf, in_=xtf)

        # pooled (mean over T), column layout
        for k in range(KH):
            nc.vector.reduce_sum(
                out=poolcol[:, k : k + 1], in_=xtf[:, k, :],
                axis=mybir.AxisListType.X,
            )
        nc.scalar.mul(poolcol, poolcol, 1.0 / T)
        nc.sync.dma_start(out=pool_o.ap(), in_=poolcol)

        # concept features transposed: cfT = Wcf @ x.T + bcf  (fp32)
        cfps = ps_g.tile([DC, T], f32, tag="sm", name="cfps")
        for k in range(KH):
            nc.tensor.matmul(
                cfps, lhsT=wcf[:, k, :], rhs=xtf[:, k, :],
                start=(k == 0), stop=(k == KH - 1),
            )
        nc.scalar.activation(out=cfT, in_=cfps, func=AF.Identity, bias=bcf, scale=1.0)
        nc.sync.dma_start(out=cf_o.ap(), in_=cfT)

        # context column (bf16): cols 0..7 = pooled, col 8 rows 0..7 = cf means
        csum = grus.tile([DC, 1], f32, tag="csum")
        nc.vector.reduce_sum(out=csum, in_=cfT, axis=mybir.AxisListType.X)
        nc.scalar.mul(csum, csum, 1.0 / T)
        nc.vector.tensor_copy(out=ctxc[:, 0:KH], in_=poolcol)
        nc.vector.tensor_copy(out=ctxc[0:DC, KH : KH + 1], in_=csum)

        # h0 = context @ W_c2h.T + b_c2h (row layout)
        for half in range(2):
            h0p = ps_g.tile([1, T], f32, tag="sm", name=f"h0p{half}")
            sl = slice(half * T, (half + 1) * T)
            for k in range(KH):
                nc.tensor.matmul(
                    h0p, lhsT=ctxc[:, k : k + 1], rhs=wc2hm[:, k, sl],
                    start=(k == 0), stop=False,
                )
            nc.tensor.matmul(
                h0p, lhsT=ctxc[0:DC, KH : KH + 1], rhs=wc2ht[:, sl],
                start=False, stop=True,
            )
            nc.vector.tensor_add(h_row[0:1, sl], h0p, bc2h[0:1, sl])

        def transpose_h():
            # h_row [1,H] -> hT_f/hT_b [P,KH] via K=1 matmuls
            trp = ps_g.tile([P, KH], f32, tag="sm", name="trp")
            for j in range(KH):
                nc.tensor.matmul(
                    trp[:, j : j + 1], lhsT=h_row[0:1, ts(j, P)], rhs=ones1,
                    start=True, stop=True,
                )
            nc.vector.tensor_copy(out=hT_f, in_=trp)
            nc.vector.tensor_copy(out=hT_b, in_=trp)

        transpose_h()

        # ---------------- expert phase A ----------------
        hm_tiles = [None] * V

        def emit_A(v):
            w1t = w1p.tile([P, KH, H], bf16, tag="w1", name=f"w1t{v}")
            nc.sync.dma_start(out=w1t, in_=w1a[v])
            hmt = hmp.tile([P, KH, T], bf16, tag="hm", name=f"hm{v}")
            hm_tiles[v] = hmt
            for m in range(MH):
                ps1 = ps_a.tile([P, T], f32, tag="a", name=f"ps1_{v}_{m}")
                for k in range(KH):
                    nc.tensor.matmul(
                        ps1, lhsT=w1t[:, k, ts(m, P)], rhs=xbf[:, k, :],
                        start=(k == 0), stop=(k == KH - 1),
                    )
                nc.scalar.activation(
                    out=hmt[:, m, :], in_=ps1, func=AF.Gelu,
                    bias=b1c[:, v, m : m + 1], scale=1.0,
                )

        # ---------------- GRU step ----------------
        def emit_gru_step(l):
            # gh = h @ W_hh.T  (+ gate math); r cols [0,H), z [H,2H), n [2H,3H)
            ghr = []
            ghn = []
            for half in range(2):
                g = ps_g.tile([1, T], f32, tag="gh", name=f"ghr{l}{half}")
                c0 = half * T
                for k in range(KH):
                    nc.tensor.matmul(
                        g, lhsT=hT_b[:, k : k + 1], rhs=whh[:, k, c0 : c0 + T],
                        start=(k == 0), stop=(k == KH - 1),
                    )
                ghr.append(g)
            for half in range(2):
                g = ps_g.tile([1, T], f32, tag="gh", name=f"ghn{l}{half}")
                c0 = 2 * H + half * T
                for k in range(KH):
                    nc.tensor.matmul(
                        g, lhsT=hT_b[:, k : k + 1], rhs=whh[:, k, c0 : c0 + T],
                        start=(k == 0), stop=(k == KH - 1),
                    )
                ghn.append(g)
            # r = sigmoid(ghr + brz[:H]); n_pre = (ghn + bhn)*r + bin -> tanh
            for half in range(2):
                sl = slice(half * T, (half + 1) * T)
                nc.vector.tensor_add(t_row[0:1, sl], ghr[half], brz[0:1, sl])
                nc.scalar.activation(r_row[0:1, sl], t_row[0:1, sl], AF.Sigmoid)
            for half in range(2):
                sl = slice(half * T, (half + 1) * T)
                nc.vector.tensor_add(t_row[0:1, sl], ghn[half], bhn[0:1, sl])
                nc.vector.tensor_mul(t_row[0:1, sl], t_row[0:1, sl], r_row[0:1, sl])
                nc.vector.tensor_add(t_row[0:1, sl], t_row[0:1, sl], bin_[0:1, sl])
                nc.scalar.activation(n_row[0:1, sl], t_row[0:1, sl], AF.Tanh)
            ghz = []
            for half in range(2):
                g = ps_g.tile([1, T], f32, tag="gh", name=f"ghz{l}{half}")
                c0 = H + half * T
                for k in range(KH):
                    nc.tensor.matmul(
                        g, lhsT=hT_b[:, k : k + 1], rhs=whh[:, k, c0 : c0 + T],
                        start=(k == 0), stop=(k == KH - 1),
                    )
                ghz.append(g)
            for half in range(2):
                sl = slice(half * T, (half + 1) * T)
                nc.vector.tensor_add(t_row[0:1, sl], ghz[half], brz[0:1, H + half * T : H + (half + 1) * T])
                nc.scalar.activation(z_row[0:1, sl], t_row[0:1, sl], AF.Sigmoid)
            # h = n + z*(h - n)
            nc.vector.tensor_sub(t_row, h_row, n_row)
            nc.vector.tensor_mul(t_row, t_row, z_row)
            nc.vector.tensor_add(h_row, n_row, t_row)

            transpose_h()

            # logits_l = h @ W_proj.T + b_proj  (fp32)
            lgp = ps_g.tile([1, V], f32, tag="sm", name=f"lgp{l}")
            for k in range(KH):
                nc.tensor.matmul(
                    lgp, lhsT=hT_f[:, k : k + 1], rhs=wpj[:, k, :],
                    start=(k == 0), stop=(k == KH - 1),
                )
            lgrow = lgrows[l]
            nc.vector.tensor_add(lgrow, lgp, bpj)
            nc.sync.dma_start(out=lg_o.ap()[l : l + 1, :], in_=lgrow)

            # softmax row, accumulate into w_work
            mx = grus.tile([1, 1], f32, tag="mx", name=f"mx{l}")
            nc.vector.tensor_reduce(
                out=mx, in_=lgrow, axis=mybir.AxisListType.X, op=AluOpType.max
            )
            nc.vector.tensor_scalar_mul(mx, mx, -1.0)
            erow = grus.tile([1, V], f32, tag="erow", name=f"erow{l}")
            nc.scalar.activation(erow, lgrow, AF.Exp, bias=mx, scale=1.0)
            sm_s = grus.tile([1, 1], f32, tag="sm_s", name=f"sms{l}")
            nc.vector.reduce_sum(out=sm_s, in_=erow, axis=mybir.AxisListType.X)
            nc.vector.reciprocal(sm_s, sm_s)
            nc.vector.tensor_scalar_mul(erow, erow, sm_s)
            if l == 0:
                nc.vector.tensor_copy(out=w_work, in_=erow)
            else:
                nc.vector.tensor_add(w_work, w_work, erow)

        def finalize_w():
            nc.vector.tensor_scalar_mul(w_row, w_work, 1.0 / L)
            wbp = ps_g.tile([P, V], f32, tag="sm", name="wbp")
            nc.tensor.matmul(wbp, lhsT=onesr, rhs=w_row, start=True, stop=True)
            nc.vector.tensor_copy(out=wb, in_=wbp)
            # b2mix = sum_v w[v] * b2[v]  (column layout)
            nc.vector.tensor_scalar_mul(b2mix, b2c[:, :, 0], wb[:, 0:1])
            for v in range(1, V):
                nc.vector.scalar_tensor_tensor(
                    out=b2mix, in0=b2c[:, :, v], scalar=wb[:, v : v + 1],
                    in1=b2mix, op0=AluOpType.mult, op1=AluOpType.add,
                )

        # interleave: experts keep PE busy while GRU serial path runs
        emit_A(0)
        emit_gru_step(0)
        emit_A(1)
        emit_gru_step(1)
        emit_A(2)
        emit_gru_step(2)
        emit_A(3)
        emit_gru_step(3)
        finalize_w()
        emit_A(4)
        emit_A(5)

        # free GRU-era pools, open phase-B pools in the freed space
        grus_cm.__exit__(None, None, None)
        ps_g_cm.__exit__(None, None, None)

        w2p = ctx.enter_context(tc.tile_pool(name="w2p", bufs=2))
        ps_b = ctx.enter_context(tc.tile_pool(name="ps_b", bufs=4, space="PSUM"))
        out_acc = w2p.tile([P, MH, T], f32, tag="oacc", bufs=1)

        def emit_B(v):
            w2t = w2p.tile([P, KH, H], bf16, tag="w2", name=f"w2t{v}")
            nc.sync.dma_start(out=w2t, in_=w2a[v])
            hmt = hm_tiles[v]
            for m2 in range(MH):
                ps2 = ps_b.tile([P, T], f32, tag="b", name=f"ps2_{v}_{m2}")
                for k2 in range(KH):
                    nc.tensor.matmul(
                        ps2, lhsT=w2t[:, k2, ts(m2, P)], rhs=hmt[:, k2, :],
                        start=(k2 == 0), stop=(k2 == KH - 1),
                    )
                if v == 0:
                    nc.vector.tensor_scalar(
                        out=out_acc[:, m2, :], in0=ps2,
                        scalar1=wb[:, 0:1], scalar2=b2mix[:, m2 : m2 + 1],
                        op0=AluOpType.mult, op1=AluOpType.add,
                    )
                else:
                    nc.vector.scalar_tensor_tensor(
                        out=out_acc[:, m2, :], in0=ps2, scalar=wb[:, v : v + 1],
                        in1=out_acc[:, m2, :],
                        op0=AluOpType.mult, op1=AluOpType.add,
                    )
                if v == V - 1:
                    nc.sync.dma_start(
                        out=outT_o.ap()[:, m2, :], in_=out_acc[:, m2, :]
                    )

        emit_B(0)
        emit_A(6)
        emit_B(1)
        emit_A(7)
        for v in range(2, V):
            emit_B(v)

    nc.compile()
    return nc


def _prep_inputs(inputs):
    """Host-side marshalling: fold concept head into W1, transpose/retile
    weights into the SBUF-friendly layouts, cast matmul operands to bf16."""
    f32 = np.float32

    x = np.asarray(inputs["x"], f32)
    W_tt = np.asarray(inputs["W_tt"], f32)
    b_tt = np.asarray(inputs["b_tt"], f32)
    W_pol = np.asarray(inputs["W_pol"], f32)
    b_pol = np.asarray(inputs["b_pol"], f32)
    W_c2h = np.asarray(inputs["W_c2h"], f32)
    b_c2h = np.asarray(inputs["b_c2h"], f32)
    W_ih = np.asarray(inputs["W_ih"], f32)
    W_hh = np.asarray(inputs["W_hh"], f32)
    b_ih = np.asarray(inputs["b_ih"], f32)
    b_hh = np.asarray(inputs["b_hh"], f32)
    W_proj = np.asarray(inputs["W_proj"], f32)
    b_proj = np.asarray(inputs["b_proj"], f32)
    W1 = np.asarray(inputs["W1"], f32)
    b1 = np.asarray(inputs["b1"], f32)
    W2 = np.asarray(inputs["W2"], f32)
    b2 = np.asarray(inputs["b2"], f32)

    Wcf = np.concatenate([W_tt, W_pol], 0)        # (8, H)
    bcf = np.concatenate([b_tt, b_pol], 0)        # (8,)

    W1x = W1[:, :, :H]
    W1c = W1[:, :, H:]                            # (V, H, 8)
    W1_eff = W1x + np.einsum("vhc,cd->vhd", W1c, Wcf).astype(f32)
    b1_eff = b1 + W1c @ bcf                       # (V, H)

    def col8(mat_t):  # (1024, N) -> (P, 8, N)
        n = mat_t.shape[1]
        return np.ascontiguousarray(
            mat_t.reshape(KH, P, n).transpose(1, 0, 2)
        )

    w1_np = np.ascontiguousarray(
        W1_eff.transpose(0, 2, 1).reshape(V, KH, P, H).transpose(0, 2, 1, 3)
    ).astype(BF16)
    w2_np = np.ascontiguousarray(
        W2.transpose(0, 2, 1).reshape(V, KH, P, H).transpose(0, 2, 1, 3)
    ).astype(BF16)

    wc2hT = W_c2h.T                               # (DIN, H)
    shared = {
        "w1": w1_np,
        "w2": w2_np,
        "wcf": col8(Wcf.T.astype(f32)),
        "bcf": bcf.reshape(DC, 1),
        "wc2hm": col8(wc2hT[:H]).astype(BF16),
        "wc2ht": np.ascontiguousarray(wc2hT[H:]).astype(BF16),
        "bc2h": b_c2h.reshape(1, H),
        "whh": col8(W_hh.T).astype(BF16),
        "brz": (b_ih[: 2 * H] + b_hh[: 2 * H]).reshape(1, 2 * H),
        "bhn": b_hh[2 * H :].reshape(1, H),
        "bin": b_ih[2 * H :].reshape(1, H),
        "wpj": col8(W_proj.T.astype(f32)),
        "bpj": b_proj.reshape(1, V),
        "b1c": np.ascontiguousarray(
            b1_eff.reshape(V, MH, P).transpose(2, 0, 1)
        ),
        "b2c": np.ascontiguousarray(b2.reshape(V, MH, P).transpose(2, 1, 0)),
    }
    shared = {
        k: (v if v.dtype == BF16 else np.ascontiguousarray(v, f32))
        for k, v in shared.items()
    }

    in_maps = []
    for i in range(NCORES):
        m = dict(shared)
        m["xt"] = np.ascontiguousarray(
            x[i].T.reshape(KH, P, T).transpose(1, 0, 2), f32
        )
        in_maps.append(m)
    return in_maps


def kernel(**inputs):
    global LAST_RESULTS
    from concourse import bass_utils

    if "nc" not in _CACHE:
        _CACHE["nc"] = _build_program()
    nc = _CACHE["nc"]

    in_maps = _prep_inputs(inputs)
    trace = os.environ.get("KERNEL_TRACE", "0") == "1"
    res = bass_utils.run_bass_kernel_spmd(
        nc, in_maps, core_ids=list(range(NCORES)), trace=trace
    )
    LAST_RESULTS = res

    out = np.empty((B, T, H), np.float32)
    logits = np.empty((B, L, V), np.float32)
    tt = np.empty((B, T, 6), np.float32)
    pol = np.empty((B, T, 2), np.float32)
    pooled = np.empty((B, H), np.float32)
    for i in range(NCORES):
        r = res.results[i]
        out[i] = r["outT"].transpose(2, 1, 0).reshape(T, H)
        logits[i] = r["lg"]
        cf = r["cf"]
        tt[i] = cf[:6].T
        pol[i] = cf[6:8].T
        pooled[i] = r["pool"].T.reshape(H)
    return out, logits, tt, pol, pooled
